# revision 1
# baseline (speedup 1.0000x reference)
"""Trainium2 Bass kernel for ExpertsChooseMaskedExpand MoE routing.

Math (reference):
    xd[b,e,c,i] = sum_t x[b,t,(e,i)] * dmask[b,t,e,c]            (dispatch)
    y[b,e,c,o]  = sum_i xd[b,e,c,i] * w[e,o,i] + bias[o]         (expert mm)
    out[b,t,o]  = sum_{e,c} y[b,e,c,o] * cmb[b,t,e,c]            (combine)

Restructured (combine applied before the weight matmul — 155 GF total
instead of 215 GF; the E expert matmuls fuse into one K=2048 matmul):
    xd[b,e][c,j] = sum_t dmask[b,e][t,c] * xr[b,e][t,j]
    zT[b,e][j,t] = sum_c xd[b,e][c,j] * cmbT[b,e][c,t]
    out[b][t,o]  = sum_{(e,j)} zT[b][(e,j),t] * wstack[(e,j),o] + s[b][t]*bias[o]
    where s[b][t] = sum_{e,c} cmb[b,t,e,c],  wstack[(e,j),o] = w[e,o,j]

Sharding: 8 cores = (batch b in 0..3) x (expert pair h in 0..1). Core
(b, h) runs dispatch+combine for experts {2h, 2h+1} only (phases 1-2,
K = 1024 of the fused contraction) and produces a partial output over
the FULL 8192 output columns; the host sums the two partials per batch
(fp32) and unpacks the o-major packing. No on-device collectives. The
bias rank-1 term s[t]*bias[o] is applied only on h=0 cores (h=1 cores
receive a zero biasT), fused into the PSUM->SBUF eviction on the
vector engine. All matmuls run as float32r (fp22, full PE rate).

Phase 3 runs transposed: stationary = weight block (j, o-tile), moving
= zT t-chunks, PSUM holds out^T (o, t).
"""

import numpy as np

B, T, E, C = 4, 1024, 4, 512
IN, OUT = 2048, 8192
P = 128
TT = T // P          # 8  t-tiles
CT = C // P          # 4  c-tiles per expert
JT = 4               # j-tiles per expert (i = 512)
EL = 2               # experts handled per core (expert-pair split)
KT = EL * JT         # 8 k-tiles for the fused matmul (K = 1024 per core)
OT = OUT // P        # 64 o-tiles of 128 (full output width per core)
TCH = 2              # t-chunks of 512

_CACHE = {}


def _build_nc():
    import concourse.mybir as mybir
    import concourse.tile as tile
    from concourse import bacc

    f32 = mybir.dt.float32
    f32r = mybir.dt.float32r

    nc = bacc.Bacc("TRN2", target_bir_lowering=False, debug=False, num_devices=8)
    x_t = nc.dram_tensor("x", (T, EL * 512), f32r, kind="ExternalInput")
    dm_t = nc.dram_tensor("dm", (T, EL, C), f32r, kind="ExternalInput")
    cT_t = nc.dram_tensor("cmbT", (EL, C, T), f32r, kind="ExternalInput")
    # wpk[p, ot, kt, oi] = wstack[h*1024 + kt*128+p, ot*128 + oi]
    wpk_t = nc.dram_tensor("wpk", (P, OT, KT, P), f32r, kind="ExternalInput")
    sb_t = nc.dram_tensor("sb", (P, T), f32, kind="ExternalInput")       # s bcast
    bT_t = nc.dram_tensor("biasT", (P, OT), f32, kind="ExternalInput")
    # out_pk[p, ot, tch, u] = out[tch*512+u, ot*128+p]
    o_t = nc.dram_tensor("out", (P, OT, TCH, 512), f32, kind="ExternalOutput")

    x_r = x_t.ap().rearrange("(tt p) f -> p tt f", p=P)        # [128, 8, 1024]
    dm_r = dm_t.ap().rearrange("(tt p) e c -> p tt e c", p=P)  # [128, 8, 2, 512]
    cT_r = cT_t.ap().rearrange("e (ct p) t -> p e ct t", p=P)  # [128, 2, 4, 1024]
    wpk_r = wpk_t.ap()                                         # [128, 64, 8, 128]
    o_r = o_t.ap()                                             # [128, 64, 2, 512]

    with tile.TileContext(nc) as tc:
        with (
            tc.tile_pool(name="persist", bufs=1) as persist,
            tc.tile_pool(name="wp", bufs=10) as wp,
            tc.tile_pool(name="op", bufs=4) as op,
        ):
            zT = persist.tile([P, KT, T], f32r)       # 64 KiB/partition
            sb_sb = persist.tile([P, T], f32)
            bT_sb = persist.tile([P, OT], f32)

            w_tiles = {}

            def load_w(ot):
                t = wp.tile([P, KT, P], f32r, tag="w", name=f"w_{ot}")
                nc.sync.dma_start(t, wpk_r[:, ot, :, :])
                w_tiles[ot] = t

            # ---- Phases 1+2: per-expert dispatch and combine ----
            with (
                tc.tile_pool(name="xdm", bufs=5) as xdm,
                tc.tile_pool(name="cp", bufs=3) as cp,
                tc.tile_pool(name="xdp", bufs=1) as xdp,
                tc.tile_pool(name="ps_a", bufs=4, space="PSUM") as ps_a,
                tc.tile_pool(name="ps_b", bufs=2, space="PSUM") as ps_b,
            ):
                for e in range(EL):
                    # phase 1: xd[c, j] = sum_t dm[t, c] * x[t, j]
                    # quarter-granularity loads; tt-outer so matmuls start early
                    xq, dmq = {}, {}
                    for qt in range(4):
                        qs = slice(qt * 2, qt * 2 + 2)
                        xq[qt] = xdm.tile(
                            [P, 2, 512], f32r, tag="x", name=f"x_{e}_{qt}"
                        )
                        dmq[qt] = xdm.tile(
                            [P, 2, 512], f32r, tag="dm", name=f"dm_{e}_{qt}"
                        )
                        if e == 0 and qt == 0:
                            # first tiles split in half so tt=0 matmuls can
                            # start on the first 0.25 MB instead of 0.5 MB
                            for hh in range(2):
                                nc.sync.dma_start(
                                    xq[qt][:, hh : hh + 1, :],
                                    x_r[:, hh : hh + 1, e * 512 : (e + 1) * 512],
                                )
                                nc.sync.dma_start(
                                    dmq[qt][:, hh : hh + 1, :],
                                    dm_r[:, hh : hh + 1, e, :],
                                )
                        else:
                            nc.sync.dma_start(
                                xq[qt], x_r[:, qs, e * 512 : (e + 1) * 512]
                            )
                            nc.sync.dma_start(dmq[qt], dm_r[:, qs, e, :])
                    # combine loads for this expert, issued before phase-1 runs
                    c_ths = []
                    for th in range(2):
                        c_th = cp.tile([P, CT, 512], f32r, tag="c", name=f"c_{e}_{th}")
                        nc.sync.dma_start(
                            c_th, cT_r[:, e, :, th * 512 : (th + 1) * 512]
                        )
                        c_ths.append(c_th)
                    ps1 = [
                        ps_a.tile([P, 512], f32, tag="ps1", name=f"ps1_{e}_{ct}")
                        for ct in range(CT)
                    ]
                    for tt in range(TT):
                        qt, qi = tt // 2, tt % 2
                        for ct in range(CT):
                            nc.tensor.matmul(
                                ps1[ct],
                                dmq[qt][:, qi, ct * P : (ct + 1) * P],
                                xq[qt][:, qi, :],
                                start=(tt == 0),
                                stop=(tt == TT - 1),
                            )
                    xd_e = xdp.tile([P, CT, 512], f32r, tag="xd")
                    for ct in range(CT):
                        nc.vector.tensor_copy(xd_e[:, ct, :], ps1[ct])

                    # phase 2: zT[j, t] = sum_c xd[c, j] * cmbT[c, t]
                    for th in range(2):
                        c_th = c_ths[th]
                        for jt in range(JT):
                            ps2 = ps_b.tile([P, 512], f32, tag="ps2")
                            for ct in range(CT):
                                nc.tensor.matmul(
                                    ps2,
                                    xd_e[:, ct, jt * P : (jt + 1) * P],
                                    c_th[:, ct, :],
                                    start=(ct == 0),
                                    stop=(ct == CT - 1),
                                )
                            nc.vector.tensor_copy(
                                zT[:, e * JT + jt, th * 512 : (th + 1) * 512], ps2
                            )

                    # weight prefetch at the tail of each expert's DMA stream
                    for ot in {0: [0, 1], 1: [2, 3]}.get(e, []):
                        load_w(ot)
                    # phase-3-only inputs, after the last expert's stream
                    if e == EL - 1:
                        nc.sync.dma_start(sb_sb, sb_t.ap())
                        nc.sync.dma_start(bT_sb, bT_t.ap())

            # ---- Phase 3 (transposed): outT[o,t] = sum_kt w[kt].T @ zT[kt] ----
            with tc.tile_pool(name="ps_c", bufs=8, space="PSUM") as ps_c:
                for ot in range(OT):
                    for pot in range(ot, min(ot + 10, OT)):
                        if pot not in w_tiles:
                            load_w(pot)
                    psum = [
                        ps_c.tile([P, 512], f32, tag="ps3", name=f"ps3_{ot}_{i}")
                        for i in range(TCH)
                    ]
                    for kt in range(KT):
                        st = w_tiles[ot][:, kt, :]
                        for tch in range(TCH):
                            nc.tensor.matmul(
                                psum[tch],
                                st,
                                zT[:, kt, tch * 512 : (tch + 1) * 512],
                                start=(kt == 0),
                                stop=(kt == KT - 1),
                            )
                    for tch in range(TCH):
                        o_sb = op.tile([P, 512], f32, tag="o_sb")
                        # outT = s_bcast[:, tch] * biasT[:, ot] + psum
                        nc.vector.scalar_tensor_tensor(
                            o_sb,
                            sb_sb[:, tch * 512 : (tch + 1) * 512],
                            bT_sb[:, ot : ot + 1],
                            psum[tch],
                            mybir.AluOpType.mult,
                            mybir.AluOpType.add,
                        )
                        nc.gpsimd.dma_start(o_r[:, ot, tch, :], o_sb)

    nc.compile()
    return nc


def _get_nc():
    if "nc" not in _CACHE:
        _CACHE["nc"] = _build_nc()
    return _CACHE["nc"]


def _prep_in_maps(x, combine_array, dispatch_mask, weight, bias):
    x = np.ascontiguousarray(x, dtype=np.float32)
    cmb = np.ascontiguousarray(combine_array, dtype=np.float32)
    dm = np.ascontiguousarray(dispatch_mask, dtype=np.float32)
    weight = np.ascontiguousarray(weight, dtype=np.float32)
    bias = np.ascontiguousarray(bias, dtype=np.float32)

    # combine transposed to (B, E, C, T) so that C lands on partitions
    cmbT = np.ascontiguousarray(cmb.transpose(0, 2, 3, 1))
    s = cmb.sum(axis=(2, 3))  # (B, T)
    sb = [np.ascontiguousarray(np.broadcast_to(s[b], (P, T))) for b in range(B)]
    # wstack[(e,j), o] = w[e, o, j];  w = weight.reshape(E, OUT, IN//E)
    w = weight.reshape(E, OUT, IN // E)
    wstack = np.ascontiguousarray(w.transpose(0, 2, 1)).reshape(IN, OUT)
    # expert-pair h owns wstack rows [h*1024, (h+1)*1024) over the full OUT
    wpk = []
    for h in range(2):
        wh = wstack[h * 1024 : (h + 1) * 1024, :].reshape(KT, P, OT, P)
        wpk.append(np.ascontiguousarray(wh.transpose(1, 2, 0, 3)))  # (p, ot, kt, oi)
    # bias applied once per pair: even cores get the real bias, odd get zeros
    bT = [
        np.ascontiguousarray(bias.reshape(OT, P).T),
        np.zeros((P, OT), dtype=np.float32),
    ]

    in_maps = []
    for k in range(8):
        b, h = k // 2, k % 2
        in_maps.append(
            {
                "x": np.ascontiguousarray(x[b][:, h * 1024 : (h + 1) * 1024]),
                "dm": np.ascontiguousarray(dm[b][:, 2 * h : 2 * h + 2, :]),
                "cmbT": np.ascontiguousarray(cmbT[b][2 * h : 2 * h + 2]),
                "wpk": wpk[h],
                "sb": sb[b],
                "biasT": bT[h],
            }
        )
    return in_maps


def _enable_persistent_cache():
    try:
        import jax

        jax.config.update("jax_compilation_cache_dir", "/tmp/jax_neff_cache")
        jax.config.update("jax_persistent_cache_min_compile_time_secs", 1.0)
    except Exception:
        pass


def run_spmd(in_maps, trace=False, **kwargs):
    from concourse.bass_utils import run_bass_kernel_spmd

    _enable_persistent_cache()
    nc = _get_nc()
    return run_bass_kernel_spmd(
        nc, in_maps, core_ids=list(range(8)), trace=trace, **kwargs
    )


def kernel(x, combine_array, dispatch_mask, weight, bias, num_experts):
    assert int(num_experts) == E
    in_maps = _prep_in_maps(x, combine_array, dispatch_mask, weight, bias)
    try:
        res = run_spmd(in_maps)
    except Exception:
        # transient device errors (e.g. a wedged core from a prior run)
        # usually clear on retry with a freshly built program
        _CACHE.clear()
        res = run_spmd(in_maps)
    out = np.empty((B, T, OUT), dtype=np.float32)
    for b in range(B):
        pk = res.results[2 * b]["out"] + res.results[2 * b + 1]["out"]
        out[b] = pk.transpose(2, 3, 1, 0).reshape(T, OUT)  # (P,OT,TCH,512)->(t,o)
    return out



# revision 3
# speedup vs baseline: 1.5565x; 1.5565x over previous
"""Trainium2 Bass kernel for ExpertsChooseMaskedExpand MoE routing.

Math (reference):
    xd[b,e,c,i] = sum_t x[b,t,(e,i)] * dmask[b,t,e,c]            (dispatch)
    y[b,e,c,o]  = sum_i xd[b,e,c,i] * w[e,o,i] + bias[o]         (expert mm)
    out[b,t,o]  = sum_{e,c} y[b,e,c,o] * cmb[b,t,e,c]            (combine)

Restructured with combine applied before the weight matmul (155 GF
instead of 215), then MEAN-SPLIT so the dominant contraction can run
in fp8 DoubleRow (2 MACs/cell/cycle) without losing accuracy:

    cmb = q + cmb'          q[b,e,c] = mean_t cmb  (host, exact)
    z   = mu + nu           mu[b,(e,i)] = sum_c xd*q  (token-independent;
                            97% of z's variance), nu from cmb'
    out[b,t,o] = nu@wstack + s'[t]*bias[o]            (device, fp8)
               + mu@wstack[o] + s_q*bias[o]           (host, exact rank-1)

mu never touches the device: mu = sum_t (dm@q)[t] * x[t,:], all host
inputs. The device-side fp8 error only scales with nu (std ~120) while
the error tolerance scales with max|out| (dominated by the mu part,
~100x larger) — measured end-to-end rel err ~2e-3 vs the 2e-2 gate.

Device phases per core (8 cores = batch b x expert-pair h):
    1. xd[c,j] = sum_t dm[t,c]*x[t,j]           f32r, per expert
    2. nuT[j,t] = sum_c xd[c,j]*cmbT'[c,t]      f32r, PSUM evicted to fp8
    3. outT[o,t] = sum_k w8[k,o]*nuT[k,t]       fp8e4 DoubleRow, K=256/MM
bias applied via s'[t]*bias[o] on h=0 cores during PSUM eviction
(vector stt), output stored bf16; host sums the two K-half partials,
rescales by 1/(ws*zs) and adds the exact rank-1 correction C[b,o].
"""

import numpy as np
import ml_dtypes

B, T, E, C = 4, 1024, 4, 512
IN, OUT = 2048, 8192
P = 128
TT = T // P          # 8  t-tiles
CT = C // P          # 4  c-tiles per expert
JT = 4               # j-tiles per expert (i = 512)
EL = 2               # experts handled per core (expert-pair split)
KT = EL * JT         # 8 k-tiles for the fused matmul (K = 1024 per core)
KTP = KT // 2        # 4 DoubleRow k-tile pairs
OT = OUT // P        # 64 o-tiles of 128 (full output width per core)
TCH = 2              # t-chunks of 512

WS = 1024.0          # weight fp8 scale (w*WS absmax ~117 < 240)
ZS = 0.0625          # nu fp8 scale (nu*ZS absmax ~95 < 240)
ALPHA = 1.0 / (WS * ZS)
N_WARM = 44          # dummy matmuls to keep HAM at K=8/8 during head DMA

_CACHE = {}


def _build_nc():
    import concourse.mybir as mybir
    import concourse.tile as tile
    from concourse import bacc

    f32 = mybir.dt.float32
    f32r = mybir.dt.float32r
    f8 = mybir.dt.float8e4
    bf16 = mybir.dt.bfloat16
    DR = mybir.MatmulPerfMode.DoubleRow

    nc = bacc.Bacc("TRN2", target_bir_lowering=False, debug=False, num_devices=8)
    x_t = nc.dram_tensor("x", (T, EL * 512), f32r, kind="ExternalInput")
    dm_t = nc.dram_tensor("dm", (T, EL, C), f32r, kind="ExternalInput")
    cT_t = nc.dram_tensor("cmbT", (EL, C, T), f32r, kind="ExternalInput")
    # w8[p, ot, m, i, oi] = fp8(WS * wstack[h*1024 + (2m+i)*128 + p, ot*128+oi])
    w8_t = nc.dram_tensor("w8", (P, OT, KTP, 2, P), f8, kind="ExternalInput")
    sb_t = nc.dram_tensor("sb", (P, T), f32, kind="ExternalInput")   # s'*WS*ZS bcast
    bT_t = nc.dram_tensor("biasT", (P, OT), f32, kind="ExternalInput")
    warm_t = nc.dram_tensor("warm", (P, 512), bf16, kind="ExternalInput")
    # out_pk[p, ot, tch, u] = bf16 of WS*ZS*(nu@w + s'*bias)[tch*512+u, ot*128+p]
    o_t = nc.dram_tensor("out", (P, OT, TCH, 512), bf16, kind="ExternalOutput")

    x_r = x_t.ap().rearrange("(tt p) f -> p tt f", p=P)        # [128, 8, 1024]
    dm_r = dm_t.ap().rearrange("(tt p) e c -> p tt e c", p=P)  # [128, 8, 2, 512]
    cT_r = cT_t.ap().rearrange("e (ct p) t -> p e ct t", p=P)  # [128, 2, 4, 1024]
    w8_r = w8_t.ap()                                           # [128, 64, 4, 2, 128]
    o_r = o_t.ap()                                             # [128, 64, 2, 512]

    with tile.TileContext(nc) as tc:
        with (
            tc.tile_pool(name="persist", bufs=1) as persist,
            tc.tile_pool(name="wp", bufs=10) as wp,
            tc.tile_pool(name="op", bufs=4) as op,
        ):
            zT = persist.tile([P, KT, T], f8)         # 8 KiB/partition
            sb_sb = persist.tile([P, T], f32)
            bT_sb = persist.tile([P, OT], f32)
            warm_sb = persist.tile([P, 512], bf16)

            w_tiles = {}

            def load_w(ot):
                t = wp.tile([P, KTP, 2, P], f8, tag="w", name=f"w_{ot}")
                nc.sync.dma_start(t, w8_r[:, ot, :, :, :])
                w_tiles[ot] = t

            # ---- Phase 0: PE warmup so HAM reaches K=8/8 before the ----
            # ---- first real matmul and phases 1-2 never run throttled ----
            with tc.tile_pool(name="wm", bufs=1, space="PSUM") as wm:
                nc.sync.dma_start(warm_sb, warm_t.ap())
                wps = wm.tile([P, 512], f32, tag="warm")
                for _ in range(N_WARM):
                    nc.tensor.matmul(
                        wps, warm_sb[:, :P], warm_sb, start=True, stop=True
                    )

            # ---- Phases 1+2: per-expert dispatch and combine ----
            with (
                tc.tile_pool(name="xdm", bufs=5) as xdm,
                tc.tile_pool(name="cp", bufs=3) as cp,
                tc.tile_pool(name="xdp", bufs=1) as xdp,
                tc.tile_pool(name="ps_a", bufs=4, space="PSUM") as ps_a,
                tc.tile_pool(name="ps_b", bufs=2, space="PSUM") as ps_b,
            ):
                for e in range(EL):
                    # phase 1: xd[c, j] = sum_t dm[t, c] * x[t, j]
                    # quarter-granularity loads; tt-outer so matmuls start early
                    xq, dmq = {}, {}
                    for qt in range(4):
                        qs = slice(qt * 2, qt * 2 + 2)
                        xq[qt] = xdm.tile(
                            [P, 2, 512], f32r, tag="x", name=f"x_{e}_{qt}"
                        )
                        dmq[qt] = xdm.tile(
                            [P, 2, 512], f32r, tag="dm", name=f"dm_{e}_{qt}"
                        )
                        if e == 0 and qt == 0:
                            # first tiles split in half so tt=0 matmuls can
                            # start on the first 0.25 MB instead of 0.5 MB
                            for hh in range(2):
                                nc.sync.dma_start(
                                    xq[qt][:, hh : hh + 1, :],
                                    x_r[:, hh : hh + 1, e * 512 : (e + 1) * 512],
                                )
                                nc.sync.dma_start(
                                    dmq[qt][:, hh : hh + 1, :],
                                    dm_r[:, hh : hh + 1, e, :],
                                )
                        else:
                            nc.sync.dma_start(
                                xq[qt], x_r[:, qs, e * 512 : (e + 1) * 512]
                            )
                            nc.sync.dma_start(dmq[qt], dm_r[:, qs, e, :])
                    # combine loads for this expert, issued before phase-1 runs
                    c_ths = []
                    for th in range(2):
                        c_th = cp.tile([P, CT, 512], f32r, tag="c", name=f"c_{e}_{th}")
                        nc.sync.dma_start(
                            c_th, cT_r[:, e, :, th * 512 : (th + 1) * 512]
                        )
                        c_ths.append(c_th)
                    ps1 = [
                        ps_a.tile([P, 512], f32, tag="ps1", name=f"ps1_{e}_{ct}")
                        for ct in range(CT)
                    ]
                    for tt in range(TT):
                        qt, qi = tt // 2, tt % 2
                        for ct in range(CT):
                            nc.tensor.matmul(
                                ps1[ct],
                                dmq[qt][:, qi, ct * P : (ct + 1) * P],
                                xq[qt][:, qi, :],
                                start=(tt == 0),
                                stop=(tt == TT - 1),
                            )
                    xd_e = xdp.tile([P, CT, 512], f32r, tag="xd")
                    for ct in range(CT):
                        nc.vector.tensor_copy(xd_e[:, ct, :], ps1[ct])

                    # phase 2: nuT[j, t] = sum_c xd[c, j] * cmbT'[c, t]
                    # PSUM evicted straight to fp8 (values pre-scaled by ZS
                    # via the host-side cmbT' scaling)
                    for th in range(2):
                        c_th = c_ths[th]
                        for jt in range(JT):
                            ps2 = ps_b.tile([P, 512], f32, tag="ps2")
                            for ct in range(CT):
                                nc.tensor.matmul(
                                    ps2,
                                    xd_e[:, ct, jt * P : (jt + 1) * P],
                                    c_th[:, ct, :],
                                    start=(ct == 0),
                                    stop=(ct == CT - 1),
                                )
                            nc.vector.tensor_copy(
                                zT[:, e * JT + jt, th * 512 : (th + 1) * 512], ps2
                            )

                    # weight prefetch at the tail of each expert's DMA stream
                    for ot in {0: [0, 1], 1: [2, 3]}.get(e, []):
                        load_w(ot)
                    # phase-3-only inputs, after the last expert's stream
                    if e == EL - 1:
                        nc.sync.dma_start(sb_sb, sb_t.ap())
                        nc.sync.dma_start(bT_sb, bT_t.ap())

            # ---- Phase 3 (fp8 DoubleRow, transposed): ----
            # ---- outT[o,t] = sum_m sum_i w8[m,i].T @ nuT[2m+i] ----
            with tc.tile_pool(name="ps_c", bufs=8, space="PSUM") as ps_c:
                for ot in range(OT):
                    for pot in range(ot, min(ot + 10, OT)):
                        if pot not in w_tiles:
                            load_w(pot)
                    psum = [
                        ps_c.tile([P, 512], f32, tag="ps3", name=f"ps3_{ot}_{i}")
                        for i in range(TCH)
                    ]
                    for m in range(KTP):
                        st = w_tiles[ot][:, m, :, :]
                        for tch in range(TCH):
                            nc.tensor.matmul(
                                psum[tch],
                                st,
                                zT[:, 2 * m : 2 * m + 2,
                                   tch * 512 : (tch + 1) * 512],
                                start=(m == 0),
                                stop=(m == KTP - 1),
                                perf_mode=DR,
                            )
                    for tch in range(TCH):
                        o_sb = op.tile([P, 512], bf16, tag="o_sb")
                        # outT = s'_scaled[:, tch] * biasT[:, ot] + psum
                        nc.vector.scalar_tensor_tensor(
                            o_sb,
                            sb_sb[:, tch * 512 : (tch + 1) * 512],
                            bT_sb[:, ot : ot + 1],
                            psum[tch],
                            mybir.AluOpType.mult,
                            mybir.AluOpType.add,
                        )
                        nc.gpsimd.dma_start(o_r[:, ot, tch, :], o_sb)

    nc.compile()
    return nc


def _get_nc():
    if "nc" not in _CACHE:
        _CACHE["nc"] = _build_nc()
    return _CACHE["nc"]


def _prep_in_maps(x, combine_array, dispatch_mask, weight, bias):
    f8 = ml_dtypes.float8_e4m3
    x = np.ascontiguousarray(x, dtype=np.float32)
    dm = np.ascontiguousarray(dispatch_mask, dtype=np.float32)
    cmb = np.asarray(combine_array, dtype=np.float64)
    weight = np.asarray(weight, dtype=np.float64)
    bias = np.asarray(bias, dtype=np.float64)

    # mean-split of the combine weights over tokens (host, exact)
    q = cmb.mean(axis=1)                           # (B, E, C)
    cmbp = cmb - q[:, None]                        # zero token-mean
    sp = cmbp.sum(axis=(2, 3))                     # (B, T)  s' for the bias term
    s_q = q.sum(axis=(1, 2))                       # (B,)
    # exact rank-1 correction: mu = sum_t (dm@q)[t]*x[t], C = mu@wstack + s_q*bias
    g = np.einsum('btec,bec->bte', dm.astype(np.float64), q)
    xr = x.astype(np.float64).reshape(B, T, E, IN // E)
    mu = np.einsum('bte,btei->bei', g, xr).reshape(B, IN)
    w_e = weight.reshape(E, OUT, IN // E)
    wstack = np.ascontiguousarray(w_e.transpose(0, 2, 1)).reshape(IN, OUT)
    corr = (mu @ wstack + s_q[:, None] * bias[None, :]).astype(np.float32)

    # combine' transposed to (B, E, C, T), pre-scaled by ZS
    cmbT = np.ascontiguousarray(
        (cmbp * ZS).transpose(0, 2, 3, 1), dtype=np.float32
    )
    sb = [
        np.ascontiguousarray(
            np.broadcast_to((sp[b] * (WS * ZS)).astype(np.float32), (P, T))
        )
        for b in range(B)
    ]
    # fp8 weights: w8[p, ot, m, i, oi] = fp8(WS*wstack[h*1024+(2m+i)*128+p, :])
    wq8 = np.clip(wstack * WS, -240.0, 240.0).astype(f8)
    w8 = []
    for h in range(2):
        wh = wq8[h * 1024 : (h + 1) * 1024, :].reshape(KT, P, OT, P)
        w8.append(
            np.ascontiguousarray(wh.transpose(1, 2, 0, 3)).reshape(P, OT, KTP, 2, P)
        )
    bT = [
        np.ascontiguousarray(bias.reshape(OT, P).T, dtype=np.float32),
        np.zeros((P, OT), dtype=np.float32),
    ]
    warm = np.zeros((P, 512), dtype=ml_dtypes.bfloat16)

    in_maps = []
    for k in range(8):
        b, h = k // 2, k % 2
        in_maps.append(
            {
                "x": np.ascontiguousarray(x[b][:, h * 1024 : (h + 1) * 1024]),
                "dm": np.ascontiguousarray(dm[b][:, 2 * h : 2 * h + 2, :]),
                "cmbT": np.ascontiguousarray(cmbT[b][2 * h : 2 * h + 2]),
                "w8": w8[h],
                "sb": sb[b],
                "biasT": bT[h],
                "warm": warm,
            }
        )
    return in_maps, corr


def _enable_persistent_cache():
    try:
        import jax

        jax.config.update("jax_compilation_cache_dir", "/tmp/jax_neff_cache")
        jax.config.update("jax_persistent_cache_min_compile_time_secs", 1.0)
    except Exception:
        pass


def run_spmd(in_maps, trace=False, **kwargs):
    from concourse.bass_utils import run_bass_kernel_spmd

    _enable_persistent_cache()
    nc = _get_nc()
    return run_bass_kernel_spmd(
        nc, in_maps, core_ids=list(range(8)), trace=trace, **kwargs
    )


def kernel(x, combine_array, dispatch_mask, weight, bias, num_experts):
    assert int(num_experts) == E
    in_maps, corr = _prep_in_maps(x, combine_array, dispatch_mask, weight, bias)
    try:
        res = run_spmd(in_maps)
    except Exception:
        # transient device errors (e.g. a wedged core from a prior run)
        # usually clear on retry with a freshly built program
        _CACHE.clear()
        res = run_spmd(in_maps)
    out = np.empty((B, T, OUT), dtype=np.float32)
    for b in range(B):
        pk = res.results[2 * b]["out"].astype(np.float32) + res.results[
            2 * b + 1
        ]["out"].astype(np.float32)
        # (P, OT, TCH, 512) -> (t, o); rescale and add exact rank-1 part
        out[b] = pk.transpose(2, 3, 1, 0).reshape(T, OUT) * ALPHA + corr[b][None, :]
    return out


# revision 6
# speedup vs baseline: 1.8717x; 1.2025x over previous
"""Trainium2 Bass kernel for ExpertsChooseMaskedExpand MoE routing.

Math (reference):
    xd[b,e,c,i] = sum_t x[b,t,(e,i)] * dmask[b,t,e,c]            (dispatch)
    y[b,e,c,o]  = sum_i xd[b,e,c,i] * w[e,o,i] + bias[o]         (expert mm)
    out[b,t,o]  = sum_{e,c} y[b,e,c,o] * cmb[b,t,e,c]            (combine)

Restructured with combine applied before the weight matmul (155 GF
instead of 215), then MEAN-SPLIT so every device matmul can run in
fp8 DoubleRow (2 MACs/cell/cycle) without losing accuracy:

    cmb = q + cmb'          q[b,e,c] = mean_t cmb  (host, exact)
    z   = mu + nu           mu[b,(e,i)] = sum_c xd*q  (token-independent;
                            97% of z's variance), nu from cmb'
    out[b,t,o] = nu@wstack                            (device, all fp8)
               + s'[t]*bias[o] + mu@wstack[o] + s_q*bias[o]   (host, exact)

mu never touches the device: mu = sum_t (dm@q)[t] * x[t,:], all host
inputs. The device-side fp8 error only scales with nu (std ~120) while
the error tolerance scales with max|out| (dominated by the mu part,
~100x larger) — measured end-to-end rel err ~3e-3 vs the 2e-2 gate.

Device phases per core (8 cores = batch b x expert-pair h), all
matmuls fp8e4 DoubleRow (K=256 per matmul):
    0. PE warmup matmuls so HAM reaches K=8/8 before real work
    1. xd[c,j] = sum_t dm8[t,c]*x8[t,j]     contraction t: 4 tt-pairs
    2. nuT[j,t] = sum_c xd8[c,j]*cmbT8[c,t] contraction c: 2 ct-pairs
    3. outT[o,t] = sum_k w8[k,o]*nuT[k,t]   contraction k: 4 kt-pairs
PSUM evictions are scale-by-pow2 copies (scalar engine for phases 1-2,
alternating vector/scalar for phase 3), output stored bf16; the host
sums the two K-half partials, rescales, and adds the exact terms.
"""

import numpy as np
import ml_dtypes

B, T, E, C = 4, 1024, 4, 512
IN, OUT = 2048, 8192
P = 128
TT = T // P          # 8  t-tiles
CT = C // P          # 4  c-tiles per expert
JT = 4               # j-tiles per expert (i = 512)
EL = 2               # experts handled per core (expert-pair split)
KT = EL * JT         # 8 k-tiles for the fused matmul (K = 1024 per core)
KTP = KT // 2        # 4 DoubleRow k-tile pairs
OT = OUT // P        # 64 o-tiles of 128 (full output width per core)
TCH = 2              # t-chunks of 512

WS = 1024.0          # weight fp8 scale   (w*WS absmax ~117 < 240)
ZS = 0.0625          # nu fp8 scale       (nu*ZS absmax ~95 < 240)
XS = 16.0            # x fp8 scale        (x*XS absmax ~90 < 240)
DS = 128.0           # dmask fp8 scale    (dm*DS < 128)
CS = 256.0           # cmb' fp8 scale     (cmb'*CS absmax ~137 < 240)
ALPHA = 1.0 / (WS * ZS)
N_WARM = 12          # keeps HAM at K=8/8 through the head DMA window

_CACHE = {}


def _build_nc():
    import concourse.mybir as mybir
    import concourse.tile as tile
    from concourse import bacc

    f32 = mybir.dt.float32
    f8 = mybir.dt.float8e4
    bf16 = mybir.dt.bfloat16
    DR = mybir.MatmulPerfMode.DoubleRow

    nc = bacc.Bacc("TRN2", target_bir_lowering=False, debug=False, num_devices=8)
    x_t = nc.dram_tensor("x", (T, EL * 512), f8, kind="ExternalInput")
    dm_t = nc.dram_tensor("dm", (T, EL, C), f8, kind="ExternalInput")
    cT_t = nc.dram_tensor("cmbT", (EL, C, T), f8, kind="ExternalInput")
    # w8[p, ot, m, i, oi] = fp8(WS * wstack[h*1024 + (2m+i)*128 + p, ot*128+oi])
    w8_t = nc.dram_tensor("w8", (P, OT, KTP, 2, P), f8, kind="ExternalInput")
    warm_t = nc.dram_tensor("warm", (P, 512), bf16, kind="ExternalInput")
    # out_pk[p, ot, tch, u] = bf16 of WS*ZS*(nu@w)[tch*512+u, ot*128+p]
    o_t = nc.dram_tensor("out", (P, OT, TCH, 512), bf16, kind="ExternalOutput")

    x_r = x_t.ap().rearrange("(tt p) f -> p tt f", p=P)        # [128, 8, 1024]
    dm_r = dm_t.ap().rearrange("(tt p) e c -> p tt e c", p=P)  # [128, 8, 2, 512]
    cT_r = cT_t.ap().rearrange("e (ct p) t -> p e ct t", p=P)  # [128, 2, 4, 1024]
    w8_r = w8_t.ap()                                           # [128, 64, 4, 2, 128]
    o_r = o_t.ap()                                             # [128, 64, 2, 512]

    with tile.TileContext(nc) as tc:
        with (
            tc.tile_pool(name="persist", bufs=1) as persist,
            tc.tile_pool(name="wp", bufs=10) as wp,
            tc.tile_pool(name="op", bufs=6) as op,
        ):
            zT = persist.tile([P, KT, T], f8)         # 8 KiB/partition
            warm_sb = persist.tile([P, 512], bf16)

            w_tiles = {}

            def load_w(ot):
                t = wp.tile([P, KTP, 2, P], f8, tag="w", name=f"w_{ot}")
                nc.sync.dma_start(t, w8_r[:, ot, :, :, :])
                w_tiles[ot] = t

            # ---- Phase 0: PE warmup so HAM reaches K=8/8 right as the ----
            # ---- first real matmul's input DMA lands (~13us) ----
            with tc.tile_pool(name="wm", bufs=1, space="PSUM") as wm:
                nc.sync.dma_start(warm_sb, warm_t.ap())
                wps = wm.tile([P, 512], f32, tag="warm")
                for _ in range(N_WARM):
                    nc.tensor.matmul(
                        wps, warm_sb[:, :P], warm_sb[:, :], start=True, stop=True
                    )

            # ---- Phases 1+2: per-expert dispatch and combine (fp8 DR) ----
            with (
                tc.tile_pool(name="xdm", bufs=5) as xdm,
                tc.tile_pool(name="cp", bufs=3) as cp,
                tc.tile_pool(name="xdp", bufs=1) as xdp,
                tc.tile_pool(name="ps_a", bufs=4, space="PSUM") as ps_a,
                tc.tile_pool(name="ps_b", bufs=2, space="PSUM") as ps_b,
            ):
                for e in range(EL):
                    # phase 1: xd[c, j] = sum_t dm[t, c] * x[t, j]
                    # quarter loads = one DoubleRow tt-pair each
                    xq, dmq = {}, {}
                    for qt in range(4):
                        qs = slice(qt * 2, qt * 2 + 2)
                        xq[qt] = xdm.tile(
                            [P, 2, 512], f8, tag="x", name=f"x_{e}_{qt}"
                        )
                        dmq[qt] = xdm.tile(
                            [P, 2, 512], f8, tag="dm", name=f"dm_{e}_{qt}"
                        )
                        if e == 0 and qt == 0:
                            # first tiles split in half so the head DMA
                            # latency is minimal
                            for hh in range(2):
                                nc.sync.dma_start(
                                    xq[qt][:, hh : hh + 1, :],
                                    x_r[:, hh : hh + 1, e * 512 : (e + 1) * 512],
                                )
                                nc.sync.dma_start(
                                    dmq[qt][:, hh : hh + 1, :],
                                    dm_r[:, hh : hh + 1, e, :],
                                )
                        else:
                            nc.sync.dma_start(
                                xq[qt], x_r[:, qs, e * 512 : (e + 1) * 512]
                            )
                            nc.sync.dma_start(dmq[qt], dm_r[:, qs, e, :])
                    # combine loads for this expert, issued before phase-1 runs
                    c_ths = []
                    for th in range(2):
                        c_th = cp.tile([P, CT, 512], f8, tag="c", name=f"c_{e}_{th}")
                        nc.sync.dma_start(
                            c_th, cT_r[:, e, :, th * 512 : (th + 1) * 512]
                        )
                        c_ths.append(c_th)
                    ps1 = [
                        ps_a.tile([P, 512], f32, tag="ps1", name=f"ps1_{e}_{ct}")
                        for ct in range(CT)
                    ]
                    for qt in range(4):        # tt-pair = DR pair
                        for ct in range(CT):
                            nc.tensor.matmul(
                                ps1[ct],
                                dmq[qt][:, :, ct * P : (ct + 1) * P],
                                xq[qt][:, :, :],
                                start=(qt == 0),
                                stop=(qt == 3),
                                perf_mode=DR,
                            )
                    # evict xd to fp8 at scale 1 (psum = XS*DS*xd)
                    xd_e = xdp.tile([P, CT, 512], f8, tag="xd")
                    for ct in range(CT):
                        nc.scalar.mul(xd_e[:, ct, :], ps1[ct], 1.0 / (XS * DS))

                    # phase 2: nuT[j, t] = sum_c xd[c, j] * cmbT'[c, t]
                    for th in range(2):
                        c_th = c_ths[th]
                        for jt in range(JT):
                            ps2 = ps_b.tile([P, 512], f32, tag="ps2")
                            for u in range(CT // 2):   # ct-pair = DR pair
                                nc.tensor.matmul(
                                    ps2,
                                    xd_e[:, 2 * u : 2 * u + 2,
                                         jt * P : (jt + 1) * P],
                                    c_th[:, 2 * u : 2 * u + 2, :],
                                    start=(u == 0),
                                    stop=(u == CT // 2 - 1),
                                    perf_mode=DR,
                                )
                            # psum = CS*nu; evict to fp8 at scale ZS
                            nc.vector.tensor_scalar_mul(
                                zT[:, e * JT + jt, th * 512 : (th + 1) * 512],
                                ps2,
                                ZS / CS,
                            )

                    # weight prefetch at the tail of each expert's DMA stream
                    for ot in {0: [0, 1], 1: [2, 3]}.get(e, []):
                        load_w(ot)

            # ---- Phase 3 (fp8 DoubleRow, transposed): ----
            # ---- outT[o,t] = sum_m sum_i w8[m,i].T @ nuT[2m+i] ----
            with tc.tile_pool(name="ps_c", bufs=8, space="PSUM") as ps_c:
                for ot in range(OT):
                    for pot in range(ot, min(ot + 10, OT)):
                        if pot not in w_tiles:
                            load_w(pot)
                    psum = [
                        ps_c.tile([P, 512], f32, tag="ps3", name=f"ps3_{ot}_{i}")
                        for i in range(TCH)
                    ]
                    for m in range(KTP):
                        st = w_tiles[ot][:, m, :, :]
                        for tch in range(TCH):
                            nc.tensor.matmul(
                                psum[tch],
                                st,
                                zT[:, 2 * m : 2 * m + 2,
                                   tch * 512 : (tch + 1) * 512],
                                start=(m == 0),
                                stop=(m == KTP - 1),
                                perf_mode=DR,
                            )
                    for tch in range(TCH):
                        o_sb = op.tile([P, 512], bf16, tag="o_sb")
                        # pure psum->bf16 copy, alternating engines so
                        # neither becomes the bottleneck
                        if (ot + tch) % 2 == 0:
                            nc.vector.tensor_copy(o_sb, psum[tch])
                        else:
                            nc.scalar.copy(o_sb, psum[tch])
                        nc.gpsimd.dma_start(o_r[:, ot, tch, :], o_sb)

    nc.compile()
    return nc


def _get_nc():
    if "nc" not in _CACHE:
        _CACHE["nc"] = _build_nc()
    return _CACHE["nc"]


def _prep_in_maps(x, combine_array, dispatch_mask, weight, bias):
    f8 = ml_dtypes.float8_e4m3
    x = np.ascontiguousarray(x, dtype=np.float32)
    dm = np.ascontiguousarray(dispatch_mask, dtype=np.float32)
    cmb = np.asarray(combine_array, dtype=np.float64)
    weight = np.asarray(weight, dtype=np.float64)
    bias = np.asarray(bias, dtype=np.float64)

    # mean-split of the combine weights over tokens (host, exact)
    q = cmb.mean(axis=1)                           # (B, E, C)
    cmbp = cmb - q[:, None]                        # zero token-mean
    sp = cmbp.sum(axis=(2, 3))                     # (B, T)  s' for the bias term
    s_q = q.sum(axis=(1, 2))                       # (B,)
    # exact corrections: mu = sum_t (dm@q)[t]*x[t]; C = mu@wstack + s_q*bias
    g = np.einsum('btec,bec->bte', dm.astype(np.float64), q)
    xr = x.astype(np.float64).reshape(B, T, E, IN // E)
    mu = np.einsum('bte,btei->bei', g, xr).reshape(B, IN)
    w_e = weight.reshape(E, OUT, IN // E)
    wstack = np.ascontiguousarray(w_e.transpose(0, 2, 1)).reshape(IN, OUT)
    corr = (mu @ wstack + s_q[:, None] * bias[None, :]).astype(np.float32)
    # full exact rank-1/2 host add-back: corr[b,o] + sp[b,t]*bias[o]
    spb = sp.astype(np.float32)
    bias32 = bias.astype(np.float32)

    def q8(a, scale):
        return np.clip(a * scale, -240.0, 240.0).astype(f8)

    x8 = q8(x, XS)                                 # (B, T, IN)
    dm8 = q8(dm, DS)                               # (B, T, E, C)
    cmbT8 = q8(np.ascontiguousarray(cmbp.transpose(0, 2, 3, 1)), CS)  # (B,E,C,T)
    wq8 = q8(wstack, WS)
    w8 = []
    for h in range(2):
        wh = wq8[h * 1024 : (h + 1) * 1024, :].reshape(KT, P, OT, P)
        w8.append(
            np.ascontiguousarray(wh.transpose(1, 2, 0, 3)).reshape(P, OT, KTP, 2, P)
        )
    warm = np.zeros((P, 512), dtype=ml_dtypes.bfloat16)

    in_maps = []
    for k in range(8):
        b, h = k // 2, k % 2
        in_maps.append(
            {
                "x": np.ascontiguousarray(x8[b][:, h * 1024 : (h + 1) * 1024]),
                "dm": np.ascontiguousarray(dm8[b][:, 2 * h : 2 * h + 2, :]),
                "cmbT": np.ascontiguousarray(cmbT8[b][2 * h : 2 * h + 2]),
                "w8": w8[h],
                "warm": warm,
            }
        )
    return in_maps, (corr, spb, bias32)


def _enable_persistent_cache():
    try:
        import jax

        jax.config.update("jax_compilation_cache_dir", "/tmp/jax_neff_cache")
        jax.config.update("jax_persistent_cache_min_compile_time_secs", 1.0)
    except Exception:
        pass


def run_spmd(in_maps, trace=False, **kwargs):
    from concourse.bass_utils import run_bass_kernel_spmd

    _enable_persistent_cache()
    nc = _get_nc()
    return run_bass_kernel_spmd(
        nc, in_maps, core_ids=list(range(8)), trace=trace, **kwargs
    )


def kernel(x, combine_array, dispatch_mask, weight, bias, num_experts):
    assert int(num_experts) == E
    in_maps, (corr, spb, bias32) = _prep_in_maps(
        x, combine_array, dispatch_mask, weight, bias
    )
    try:
        res = run_spmd(in_maps)
    except Exception:
        # transient device errors (e.g. a wedged core from a prior run)
        # usually clear on retry with a freshly built program
        _CACHE.clear()
        res = run_spmd(in_maps)
    out = np.empty((B, T, OUT), dtype=np.float32)
    for b in range(B):
        pk = res.results[2 * b]["out"].astype(np.float32) + res.results[
            2 * b + 1
        ]["out"].astype(np.float32)
        # (P, OT, TCH, 512) -> (t, o); rescale, add exact host terms
        out[b] = (
            pk.transpose(2, 3, 1, 0).reshape(T, OUT) * ALPHA
            + spb[b][:, None] * bias32[None, :]
            + corr[b][None, :]
        )
    return out
